# revision 6
# baseline (speedup 1.0000x reference)
"""Trainium2 Bass kernel for nn_DiffusionEngine (10-step gated diffusion on 19x19 boards).

Math (validated against the reference):
  a  = sigmoid(alpha); bt = softplus(beta); g = sigmoid(gamma)
  A  = (1-a-a/8) * I_361 + (a/8) * (T (x) T)   with T = 19x19 tridiagonal ones
  per step:
     b1 = A b + bt*bsrc ; w1 = A w + bt*wsrc
     b2 = relu(b1 - g*w1*bsrc) ; w2 = relu(w1 - g*b2*wsrc)
  snapshots at t in {2,5,10}; k_new = 0.7*k + ko

Device layout: fields stored as [position (3 tiles: 128/128/105), boards] per core.
A applied as 7 banded 128x128 blocks on the TensorEngine; bt*src and the gating
correction accumulate into PSUM via diag/identity matmuls; masked gating products
on the VectorEngine (scalar_tensor_tensor); relu eviction on the ScalarEngine.
Boards transposed in/out via PE transpose. k_new on GpSimd.
"""

import math

import numpy as np

NCORES = 8
B_TOTAL = 32768
PC = B_TOTAL // NCORES          # boards per core
NCHUNK = 8
CB = PC // NCHUNK               # boards per chunk (512)
NPOS = 361
TILES = ((0, 128), (128, 128), (256, 105))   # (start, size) position tiles
# banded blocks (mt, kt) of A that are nonzero
BLOCKS = [(0, 0), (0, 1), (1, 0), (1, 1), (1, 2), (2, 1), (2, 2)]
MT_KTS = {0: (0, 1), 1: (0, 1, 2), 2: (1, 2)}
SNAP_STEPS = (2, 5, 10)
RHO = 0.7

_CACHE = {}


def _build_constants(a, bt):
    """Host-side stationary matrices, packed in SBUF layout [128, n*128]."""
    c0 = 1.0 - a - a / 8.0
    c2 = a / 8.0
    T = np.zeros((19, 19), np.float64)
    for i in range(19):
        for j in range(max(0, i - 1), min(19, i + 2)):
            T[i, j] = 1.0
    A = c0 * np.eye(NPOS) + c2 * np.kron(T, T)
    W = np.zeros((128, len(BLOCKS) * 128), np.float32)
    for i, (mt, kt) in enumerate(BLOCKS):
        m0, msz = TILES[mt]
        k0, ksz = TILES[kt]
        # lhsT = [K, M] = A[kt_range, mt_range]
        W[:ksz, i * 128:i * 128 + msz] = A[k0:k0 + ksz, m0:m0 + msz].astype(np.float32)
    cst = np.zeros((128, 256), np.float32)
    cst[:, 0:128] = bt * np.eye(128, dtype=np.float32)
    cst[:, 128:256] = np.eye(128, dtype=np.float32)
    return W, cst


def _build_program(g_gate):
    import concourse.bass as bass  # noqa: F401
    import concourse.mybir as mybir
    from concourse import bacc
    from concourse.tile import TileContext

    f32 = mybir.dt.float32
    u8 = mybir.dt.uint8
    Relu = mybir.ActivationFunctionType.Relu
    mult = mybir.AluOpType.mult
    add = mybir.AluOpType.add

    nc = bacc.Bacc()
    board_d = nc.declare_dram_parameter("board", [PC, NPOS], f32, isOutput=False)
    kf_d = nc.declare_dram_parameter("kf", [PC, NPOS], f32, isOutput=False)
    ko_d = nc.declare_dram_parameter("ko", [PC, NPOS], u8, isOutput=False)
    w_d = nc.declare_dram_parameter("W", [128, len(BLOCKS) * 128], f32, isOutput=False)
    cst_d = nc.declare_dram_parameter("cst", [128, 256], f32, isOutput=False)
    phib_d = nc.declare_dram_parameter("phib", [PC, 3, NPOS], f32, isOutput=True)
    phiw_d = nc.declare_dram_parameter("phiw", [PC, 3, NPOS], f32, isOutput=True)
    knew_d = nc.declare_dram_parameter("knew", [PC, NPOS], f32, isOutput=True)

    widx = {blk: i for i, blk in enumerate(BLOCKS)}

    with TileContext(nc) as tc:
        with tc.tile_pool(name="const", bufs=1) as cpool, \
             tc.tile_pool(name="state", bufs=2) as spool, \
             tc.tile_pool(name="work", bufs=2) as wpool, \
             tc.tile_pool(name="io", bufs=3) as iopool, \
             tc.tile_pool(name="psum", bufs=1, space="PSUM") as ppool:

            wt = cpool.tile([128, len(BLOCKS) * 128], f32, name="wt")
            nc.sync.dma_start(out=wt, in_=w_d[:, :])
            cstt = cpool.tile([128, 256], f32, name="cstt")
            nc.sync.dma_start(out=cstt, in_=cst_d[:, :])
            btI = cstt[:, 0:128]
            ident = cstt[:, 128:256]

            for c in range(NCHUNK):
                r0 = c * CB

                # ---- input: board -> transposed position-major tiles ----
                stg = []
                for gi in range(4):
                    s = iopool.tile([128, NPOS], f32, tag="stg", bufs=8,
                                    name=f"stg_c{c}_g{gi}")
                    nc.sync.dma_start(
                        out=s, in_=board_d[r0 + gi * 128: r0 + (gi + 1) * 128, :])
                    stg.append(s)

                xb, xw, bsrc, wsrc = {}, {}, {}, {}
                for kt, (k0, ksz) in enumerate(TILES):
                    pt = ppool.tile([128, CB], f32, tag="pxtra", bufs=2,
                                    name=f"pt_c{c}_k{kt}")
                    for gi in range(4):
                        nc.tensor.transpose(
                            pt[:ksz, gi * 128:(gi + 1) * 128],
                            stg[gi][:, k0:k0 + ksz],
                            ident[:128, :128])
                    bs = spool.tile([128, CB], f32, tag=f"bsrc{kt}",
                                    name=f"bsrc_c{c}_k{kt}")
                    ws = spool.tile([128, CB], f32, tag=f"wsrc{kt}",
                                    name=f"wsrc_c{c}_k{kt}")
                    nc.scalar.activation(bs[:ksz], pt[:ksz], Relu)
                    nc.scalar.activation(ws[:ksz], pt[:ksz], Relu, scale=-1.0)
                    xbt = spool.tile([128, CB], f32, tag=f"xb{kt}",
                                     name=f"xb_c{c}_k{kt}")
                    xwt = spool.tile([128, CB], f32, tag=f"xw{kt}",
                                     name=f"xw_c{c}_k{kt}")
                    nc.vector.tensor_copy(xbt[:ksz], bs[:ksz])
                    nc.vector.tensor_copy(xwt[:ksz], ws[:ksz])
                    bsrc[kt], wsrc[kt], xb[kt], xw[kt] = bs, ws, xbt, xwt

                # ---- k_new (independent; GpSimd) ----
                for gi in range(4):
                    rows = slice(r0 + gi * 128, r0 + (gi + 1) * 128)
                    kft = iopool.tile([128, NPOS], f32, tag="kft",
                                      name=f"kft_c{c}_g{gi}")
                    kot = iopool.tile([128, NPOS], u8, tag="kot",
                                      name=f"kot_c{c}_g{gi}")
                    nc.sync.dma_start(out=kft, in_=kf_d[rows, :])
                    nc.sync.dma_start(out=kot, in_=ko_d[rows, :])
                    kof = iopool.tile([128, NPOS], f32, tag="kof",
                                      name=f"kof_c{c}_g{gi}")
                    nc.gpsimd.tensor_copy(kof, kot)
                    knt = iopool.tile([128, NPOS], f32, tag="knt",
                                      name=f"knt_c{c}_g{gi}")
                    nc.vector.scalar_tensor_tensor(knt, kft, RHO, kof, mult, add)
                    nc.sync.dma_start(out=knew_d[rows, :], in_=knt)

                # ---- 10 diffusion steps ----
                snap_i = 0
                for t in range(1, 11):
                    # w1 = A xw + bt*wsrc  (pre-gate w field)
                    pw = []
                    for mt, (m0, msz) in enumerate(TILES):
                        p = ppool.tile([128, CB], f32, tag=f"pw{mt}",
                                       name=f"pw_c{c}_t{t}_m{mt}")
                        kts = MT_KTS[mt]
                        for j, kt in enumerate(kts):
                            k0, ksz = TILES[kt]
                            i = widx[(mt, kt)]
                            nc.tensor.matmul(
                                p[:msz], lhsT=wt[:ksz, i * 128:i * 128 + msz],
                                rhs=xw[kt][:ksz], start=(j == 0), stop=False)
                        nc.tensor.matmul(
                            p[:msz], lhsT=btI[:msz, :msz], rhs=wsrc[mt][:msz],
                            start=False, stop=True)
                        pw.append(p)

                    # t_b = (-g * w1) .* bsrc   (DVE, reads PSUM)
                    tb = []
                    for mt, (m0, msz) in enumerate(TILES):
                        tt = wpool.tile([128, CB], f32, tag=f"tb{mt}",
                                        name=f"tb_c{c}_t{t}_m{mt}")
                        nc.vector.scalar_tensor_tensor(
                            tt[:msz], pw[mt][:msz], -g_gate, bsrc[mt][:msz],
                            mult, mult)
                        tb.append(tt)

                    # z_b = A xb + bt*bsrc + I @ t_b  (PE accumulation)
                    pb = []
                    for mt, (m0, msz) in enumerate(TILES):
                        p = ppool.tile([128, CB], f32, tag=f"pb{mt}",
                                       name=f"pb_c{c}_t{t}_m{mt}")
                        kts = MT_KTS[mt]
                        for j, kt in enumerate(kts):
                            k0, ksz = TILES[kt]
                            i = widx[(mt, kt)]
                            nc.tensor.matmul(
                                p[:msz], lhsT=wt[:ksz, i * 128:i * 128 + msz],
                                rhs=xb[kt][:ksz], start=(j == 0), stop=False)
                        nc.tensor.matmul(
                            p[:msz], lhsT=btI[:msz, :msz], rhs=bsrc[mt][:msz],
                            start=False, stop=False)
                        nc.tensor.matmul(
                            p[:msz], lhsT=ident[:msz, :msz], rhs=tb[mt][:msz],
                            start=False, stop=True)
                        pb.append(p)

                    # b2 = relu(z_b)  (ACT eviction)
                    xb_new = {}
                    for mt, (m0, msz) in enumerate(TILES):
                        xn = spool.tile([128, CB], f32, tag=f"xb{mt}",
                                        name=f"xb_c{c}_t{t}_m{mt}")
                        nc.scalar.activation(xn[:msz], pb[mt][:msz], Relu)
                        xb_new[mt] = xn

                    # z_w = w1 + (-g * b2) .* wsrc ; w2 = relu(z_w)
                    xw_new = {}
                    for mt, (m0, msz) in enumerate(TILES):
                        tw = wpool.tile([128, CB], f32, tag=f"tw{mt}",
                                        name=f"tw_c{c}_t{t}_m{mt}")
                        nc.vector.scalar_tensor_tensor(
                            tw[:msz], xb_new[mt][:msz], -g_gate, wsrc[mt][:msz],
                            mult, mult)
                        zw = wpool.tile([128, CB], f32, tag=f"zw{mt}",
                                        name=f"zw_c{c}_t{t}_m{mt}")
                        nc.vector.scalar_tensor_tensor(
                            zw[:msz], tw[:msz], 1.0, pw[mt][:msz], mult, add)
                        xn = spool.tile([128, CB], f32, tag=f"xw{mt}",
                                        name=f"xw_c{c}_t{t}_m{mt}")
                        nc.scalar.activation(xn[:msz], zw[:msz], Relu)
                        xw_new[mt] = xn

                    xb, xw = xb_new, xw_new

                    # ---- snapshots ----
                    if t in SNAP_STEPS:
                        for field, out_d, fname in ((xb, phib_d, "b"),
                                                    (xw, phiw_d, "w")):
                            for gi in range(4):
                                ps = ppool.tile([128, CB], f32, tag="pxtra",
                                                bufs=2,
                                                name=f"ps_c{c}_t{t}{fname}{gi}")
                                for kt, (k0, ksz) in enumerate(TILES):
                                    nc.tensor.transpose(
                                        ps[:128, k0:k0 + ksz],
                                        field[kt][:ksz, gi * 128:(gi + 1) * 128],
                                        ident[:ksz, :ksz])
                                sn = iopool.tile([128, NPOS], f32, tag="snap",
                                                 name=f"sn_c{c}_t{t}{fname}{gi}")
                                nc.vector.tensor_copy(sn, ps[:, 0:NPOS])
                                nc.sync.dma_start(
                                    out=out_d[r0 + gi * 128: r0 + (gi + 1) * 128,
                                              snap_i, :],
                                    in_=sn)
                        snap_i += 1
    nc.compile()
    return nc


def _get_program(a, bt, g):
    key = (round(a, 10), round(bt, 10), round(g, 10))
    if key not in _CACHE:
        W, cst = _build_constants(a, bt)
        nc = _build_program(g)
        _CACHE[key] = (nc, W, cst)
    return _CACHE[key]


def kernel(board, k_field, ko_positions, alpha, beta, gamma):
    from concourse.bass_utils import run_bass_kernel_spmd

    alpha = float(np.asarray(alpha))
    beta = float(np.asarray(beta))
    gamma = float(np.asarray(gamma))
    a = 1.0 / (1.0 + math.exp(-alpha))
    bt = math.log1p(math.exp(beta))
    g = 1.0 / (1.0 + math.exp(-gamma))

    nc, W, cst = _get_program(a, bt, g)

    board_f = np.ascontiguousarray(
        np.asarray(board, dtype=np.float32).reshape(B_TOTAL, NPOS))
    kf_f = np.ascontiguousarray(
        np.asarray(k_field, dtype=np.float32).reshape(B_TOTAL, NPOS))
    ko_u = np.ascontiguousarray(
        np.asarray(ko_positions).reshape(B_TOTAL, NPOS).astype(np.uint8))

    in_maps = []
    for core in range(NCORES):
        sl = slice(core * PC, (core + 1) * PC)
        in_maps.append({
            "board": board_f[sl],
            "kf": kf_f[sl],
            "ko": ko_u[sl],
            "W": W,
            "cst": cst,
        })

    res = run_bass_kernel_spmd(nc, in_maps, core_ids=list(range(NCORES)))
    outs = res.results

    phib = np.concatenate([outs[i]["phib"] for i in range(NCORES)], axis=0)
    phiw = np.concatenate([outs[i]["phiw"] for i in range(NCORES)], axis=0)
    knew = np.concatenate([outs[i]["knew"] for i in range(NCORES)], axis=0)

    phib = phib.reshape(B_TOTAL, 3, 19, 19)
    phiw = phiw.reshape(B_TOTAL, 3, 19, 19)
    knew = knew.reshape(B_TOTAL, 1, 19, 19)
    return phib, phiw, knew


# revision 13
# speedup vs baseline: 39.8102x; 39.8102x over previous
"""Trainium2 Bass kernel for nn_DiffusionEngine (10-step gated diffusion on 19x19 boards).

Math (validated against the reference):
  a  = sigmoid(alpha); bt = softplus(beta); g = sigmoid(gamma)
  A  = (1-a-a/8) * I_361 + (a/8) * (T (x) T)   with T = 19x19 tridiagonal ones
  per step:
     b1 = A b + bt*bsrc ; w1 = A w + bt*wsrc
     b2 = relu(b1 - g*w1*bsrc) ; w2 = relu(w1 - g*b2*wsrc)
  snapshots at t in {2,5,10}; k_new = 0.7*k + ko

Because bsrc and wsrc have disjoint support (a point is black or white, not
both), the full per-field correction collapses to v = src .* (bt - g*x):
  z_b = A b + bsrc.*(bt - g*(A w))      (cross term bsrc.*wsrc*bt vanishes)
  z_w = A w + wsrc.*(bt - g*b2)
Device mapping per step and field:
  PE:   7 banded 128x128 blocks of A (fp32, N=512) + 3 identity-matmul
        accumulates of v into PSUM
  DVE:  s = (x * -g) + bt  (tensor_scalar dual op; reads PSUM for the b side)
  Pool: v = s .* src       (tensor_tensor mult, SBUF only)
  ACT:  relu eviction PSUM -> SBUF
Fields live as fused [128, 3*512] tiles: (pos_tile, board) on the free axis.
Board in / snapshots out via PE transpose. k_new = 0.7*k+ko on Pool/DVE.
"""

import math

import numpy as np

NCORES = 8
B_TOTAL = 32768
PC = B_TOTAL // NCORES          # boards per core
NCHUNK = 8
CB = PC // NCHUNK               # boards per chunk (512)
NPOS = 361
TILES = ((0, 128), (128, 128), (256, 105))   # (start, size) position tiles
BLOCKS = [(0, 0), (0, 1), (1, 0), (1, 1), (1, 2), (2, 1), (2, 2)]
MT_KTS = {0: (0, 1), 1: (0, 1, 2), 2: (1, 2)}
SNAP_STEPS = (2, 5, 10)
RHO = 0.7

_CACHE = {}
IDENT = np.eye(128, dtype=np.float32)


def _build_constants(a):
    """Host-side stationary matrices, packed in SBUF layout [128, n*128]."""
    c0 = 1.0 - a - a / 8.0
    c2 = a / 8.0
    T = np.zeros((19, 19), np.float64)
    for i in range(19):
        for j in range(max(0, i - 1), min(19, i + 2)):
            T[i, j] = 1.0
    A = c0 * np.eye(NPOS) + c2 * np.kron(T, T)
    W = np.zeros((128, (len(BLOCKS) + 1) * 128), np.float32)
    for i, (mt, kt) in enumerate(BLOCKS):
        m0, msz = TILES[mt]
        k0, ksz = TILES[kt]
        # lhsT = [K, M] = A[kt_range, mt_range]
        W[:ksz, i * 128:i * 128 + msz] = A[k0:k0 + ksz, m0:m0 + msz].astype(np.float32)
    W[:, len(BLOCKS) * 128:] = np.eye(128, dtype=np.float32)
    return W


def _split(op, *aps):
    """Run op on [128, 0:1024] and [0:105, 1024:1536] to skip dead rows."""
    op(*[ap[:, 0:1024] for ap in aps])
    op(*[ap[0:105, 1024:1536] for ap in aps])


def _build_program(g_gate, bt):
    import concourse.bass as bass  # noqa: F401
    import concourse.mybir as mybir
    from concourse import bacc
    from concourse.tile import TileContext

    f32 = mybir.dt.float32
    f32r = mybir.dt.float32r
    u8 = mybir.dt.uint8
    Relu = mybir.ActivationFunctionType.Relu
    mult = mybir.AluOpType.mult
    add = mybir.AluOpType.add

    nc = bacc.Bacc()
    board_d = nc.declare_dram_parameter("board", [PC, NPOS], f32, isOutput=False)
    kf_d = nc.declare_dram_parameter("kf", [PC, NPOS], f32, isOutput=False)
    ko_d = nc.declare_dram_parameter("ko", [PC, NPOS], u8, isOutput=False)
    w_d = nc.declare_dram_parameter("W", [128, (len(BLOCKS) + 1) * 128], f32r,
                                    isOutput=False)
    id_d = nc.declare_dram_parameter("ident", [128, 128], f32, isOutput=False)
    phib_d = nc.declare_dram_parameter("phib", [PC, 3, NPOS], f32, isOutput=True)
    phiw_d = nc.declare_dram_parameter("phiw", [PC, 3, NPOS], f32, isOutput=True)
    knew_d = nc.declare_dram_parameter("knew", [PC, NPOS], f32, isOutput=True)

    widx = {blk: i for i, blk in enumerate(BLOCKS)}
    NI = len(BLOCKS) * 128  # identity block offset in W

    FB = 3 * CB  # fused free size (1536)

    with TileContext(nc) as tc:
        with tc.tile_pool(name="const", bufs=1) as cpool, \
             tc.tile_pool(name="state", bufs=2) as spool, \
             tc.tile_pool(name="work", bufs=1) as wpool, \
             tc.tile_pool(name="io", bufs=2) as iopool, \
             tc.tile_pool(name="psum", bufs=1, space="PSUM") as ppool:

            wt = cpool.tile([128, (len(BLOCKS) + 1) * 128], f32r, name="wt")
            nc.sync.dma_start(out=wt, in_=w_d[:, :])
            identr = wt[:, NI:NI + 128]
            ident = cpool.tile([128, 128], f32, name="ident")
            nc.sync.dma_start(out=ident, in_=id_d[:, :])

            for c in range(NCHUNK):
                r0 = c * CB

                # ---- input: board rows -> stg -> transpose -> derive ----
                stg = iopool.tile([128, 4 * NPOS], f32, tag="stg",
                                  name=f"stg_c{c}")
                nc.sync.dma_start(
                    out=stg.rearrange("p (g x) -> p g x", g=4),
                    in_=board_d[r0:r0 + CB, :].rearrange(
                        "(g p) x -> p g x", p=128))

                bsrc = spool.tile([128, FB], f32r, tag="bsrc", name=f"bsrc_c{c}")
                wsrc = spool.tile([128, FB], f32r, tag="wsrc", name=f"wsrc_c{c}")
                xb = spool.tile([128, FB], f32r, tag="xb", name=f"xb_c{c}")
                xw = spool.tile([128, FB], f32r, tag="xw", name=f"xw_c{c}")
                for kt, (k0, ksz) in enumerate(TILES):
                    pt = ppool.tile([128, CB], f32, tag="pxtra", bufs=2,
                                    name=f"pt_c{c}_k{kt}")
                    for gi in range(4):
                        nc.tensor.transpose(
                            pt[:ksz, gi * 128:(gi + 1) * 128],
                            stg[:, gi * NPOS + k0: gi * NPOS + k0 + ksz],
                            ident[:128, :128])
                    sl = slice(kt * CB, (kt + 1) * CB)
                    nc.scalar.activation(bsrc[:ksz, sl], pt[:ksz], Relu)
                    nc.scalar.activation(wsrc[:ksz, sl], pt[:ksz], Relu,
                                         scale=-1.0)
                _split(nc.vector.tensor_copy, xb, bsrc)
                _split(nc.vector.tensor_copy, xw, wsrc)

                # ---- k_new (independent; Pool cast + DVE fused op) ----
                kft = iopool.tile([128, 4 * NPOS], f32, tag="kft",
                                  name=f"kft_c{c}")
                kot = iopool.tile([128, 4 * NPOS], u8, tag="kot",
                                  name=f"kot_c{c}")
                karr = kf_d[r0:r0 + CB, :].rearrange("(g p) x -> p g x", p=128)
                koarr = ko_d[r0:r0 + CB, :].rearrange("(g p) x -> p g x", p=128)
                nc.sync.dma_start(out=kft.rearrange("p (g x) -> p g x", g=4),
                                  in_=karr)
                nc.sync.dma_start(out=kot.rearrange("p (g x) -> p g x", g=4),
                                  in_=koarr)
                kof = iopool.tile([128, 4 * NPOS], f32, tag="kof",
                                  name=f"kof_c{c}")
                nc.gpsimd.tensor_copy(kof, kot)
                knt = iopool.tile([128, 4 * NPOS], f32, tag="knt",
                                  name=f"knt_c{c}")
                nc.vector.scalar_tensor_tensor(knt, kft, RHO, kof, mult, add)
                nc.sync.dma_start(
                    out=knew_d[r0:r0 + CB, :].rearrange("(g p) x -> p g x",
                                                        p=128),
                    in_=knt.rearrange("p (g x) -> p g x", g=4))

                # ---- 10 diffusion steps ----
                snap_i = 0
                for t in range(1, 11):
                    # pw_raw = A xw (banded only)
                    pw = ppool.tile([128, FB], f32, tag="pw",
                                    name=f"pw_c{c}_t{t}")
                    for mt, (m0, msz) in enumerate(TILES):
                        kts = MT_KTS[mt]
                        for j, kt in enumerate(kts):
                            k0, ksz = TILES[kt]
                            i = widx[(mt, kt)]
                            nc.tensor.matmul(
                                pw[:msz, mt * CB:(mt + 1) * CB],
                                lhsT=wt[:ksz, i * 128:i * 128 + msz],
                                rhs=xw[:ksz, kt * CB:(kt + 1) * CB],
                                start=(j == 0), stop=False)

                    # s_b = -g * pw_raw + bt   (DVE, PSUM read)
                    sb = wpool.tile([128, FB], f32r, tag="sb",
                                    name=f"sb_c{c}_t{t}")
                    _split(lambda o, i: nc.vector.tensor_scalar(
                        o, i, -g_gate, bt, mult, add), sb, pw)
                    # v_b = s_b .* bsrc  (Pool)
                    vb = wpool.tile([128, FB], f32r, tag="vb",
                                    name=f"vb_c{c}_t{t}")
                    _split(lambda o, i0, i1: nc.gpsimd.tensor_tensor(
                        o, i0, i1, mult), vb, sb, bsrc)

                    # z_b = A xb + I @ v_b
                    pb = ppool.tile([128, FB], f32, tag="pb",
                                    name=f"pb_c{c}_t{t}")
                    for mt, (m0, msz) in enumerate(TILES):
                        kts = MT_KTS[mt]
                        for j, kt in enumerate(kts):
                            k0, ksz = TILES[kt]
                            i = widx[(mt, kt)]
                            nc.tensor.matmul(
                                pb[:msz, mt * CB:(mt + 1) * CB],
                                lhsT=wt[:ksz, i * 128:i * 128 + msz],
                                rhs=xb[:ksz, kt * CB:(kt + 1) * CB],
                                start=(j == 0), stop=False)
                        nc.tensor.matmul(
                            pb[:msz, mt * CB:(mt + 1) * CB],
                            lhsT=wt[:msz, NI:NI + msz],
                            rhs=vb[:msz, mt * CB:(mt + 1) * CB],
                            start=False, stop=True)

                    # b2 = relu(z_b)
                    xb_new = spool.tile([128, FB], f32r, tag="xb",
                                        name=f"xb_c{c}_t{t}")
                    _split(lambda o, i: nc.scalar.activation(o, i, Relu),
                           xb_new, pb)

                    # s_w = -g * b2 + bt ; v_w = s_w .* wsrc
                    sw = wpool.tile([128, FB], f32r, tag="sw",
                                    name=f"sw_c{c}_t{t}")
                    _split(lambda o, i: nc.vector.tensor_scalar(
                        o, i, -g_gate, bt, mult, add), sw, xb_new)
                    vw = wpool.tile([128, FB], f32r, tag="vw",
                                    name=f"vw_c{c}_t{t}")
                    _split(lambda o, i0, i1: nc.gpsimd.tensor_tensor(
                        o, i0, i1, mult), vw, sw, wsrc)

                    # z_w = pw_raw + I @ v_w ; w2 = relu(z_w)
                    for mt, (m0, msz) in enumerate(TILES):
                        nc.tensor.matmul(
                            pw[:msz, mt * CB:(mt + 1) * CB],
                            lhsT=wt[:msz, NI:NI + msz],
                            rhs=vw[:msz, mt * CB:(mt + 1) * CB],
                            start=False, stop=True)
                    xw_new = spool.tile([128, FB], f32r, tag="xw",
                                        name=f"xw_c{c}_t{t}")
                    _split(lambda o, i: nc.scalar.activation(o, i, Relu),
                           xw_new, pw)

                    xb, xw = xb_new, xw_new

                    # ---- snapshots ----
                    if t in SNAP_STEPS:
                        for fi, (field, out_d, fname) in enumerate(
                                ((xb, phib_d, "b"), (xw, phiw_d, "w"))):
                            for gi in range(4):
                                ps = ppool.tile([128, CB], f32, tag="pxtra",
                                                bufs=2,
                                                name=f"ps_c{c}_t{t}{fname}{gi}")
                                for kt, (k0, ksz) in enumerate(TILES):
                                    nc.tensor.transpose(
                                        ps[:128, k0:k0 + ksz],
                                        field[:ksz,
                                              kt * CB + gi * 128:
                                              kt * CB + (gi + 1) * 128
                                              ].bitcast(f32),
                                        ident[:ksz, :ksz])
                                sn = iopool.tile([128, NPOS], f32, tag="snap",
                                                 bufs=4,
                                                 name=f"sn_c{c}_t{t}{fname}{gi}")
                                # alternate evict engine to balance DVE/ACT
                                if (gi + fi) % 2 == 0:
                                    nc.vector.tensor_copy(sn, ps[:, 0:NPOS])
                                else:
                                    nc.scalar.copy(sn, ps[:, 0:NPOS])
                                nc.sync.dma_start(
                                    out=out_d[r0 + gi * 128: r0 + (gi + 1) * 128,
                                              snap_i, :],
                                    in_=sn)
                        snap_i += 1
    nc.compile()
    return nc


def _get_program(a, bt, g):
    key = (round(a, 10), round(bt, 10), round(g, 10))
    if key not in _CACHE:
        W = _build_constants(a)
        nc = _build_program(g, bt)
        _CACHE[key] = (nc, W)
    return _CACHE[key]


def kernel(board, k_field, ko_positions, alpha, beta, gamma):
    from concourse.bass_utils import run_bass_kernel_spmd

    alpha = float(np.asarray(alpha))
    beta = float(np.asarray(beta))
    gamma = float(np.asarray(gamma))
    a = 1.0 / (1.0 + math.exp(-alpha))
    bt = math.log1p(math.exp(beta))
    g = 1.0 / (1.0 + math.exp(-gamma))

    nc, W = _get_program(a, bt, g)

    board_f = np.ascontiguousarray(
        np.asarray(board, dtype=np.float32).reshape(B_TOTAL, NPOS))
    kf_f = np.ascontiguousarray(
        np.asarray(k_field, dtype=np.float32).reshape(B_TOTAL, NPOS))
    ko_u = np.ascontiguousarray(
        np.asarray(ko_positions).reshape(B_TOTAL, NPOS).astype(np.uint8))

    in_maps = []
    for core in range(NCORES):
        sl = slice(core * PC, (core + 1) * PC)
        in_maps.append({
            "board": board_f[sl],
            "kf": kf_f[sl],
            "ko": ko_u[sl],
            "W": W,
            "ident": IDENT,
        })

    res = run_bass_kernel_spmd(nc, in_maps, core_ids=list(range(NCORES)))
    outs = res.results

    phib = np.concatenate([outs[i]["phib"] for i in range(NCORES)], axis=0)
    phiw = np.concatenate([outs[i]["phiw"] for i in range(NCORES)], axis=0)
    knew = np.concatenate([outs[i]["knew"] for i in range(NCORES)], axis=0)

    phib = phib.reshape(B_TOTAL, 3, 19, 19)
    phiw = phiw.reshape(B_TOTAL, 3, 19, 19)
    knew = knew.reshape(B_TOTAL, 1, 19, 19)
    return phib, phiw, knew
